# revision 11
# baseline (speedup 1.0000x reference)
"""Grouped per-channel Linear + ReLU on 8 TRN2 NeuronCores.

Problem: out[b,c,e] = relu(sum_s x[b,s,c] * W[c,s,e] + bias[c,e])
  x: (256, 2048, 32) f32, W: (32, 2048, 2048) f32, bias: (32, 2048) f32
  out: (256, 32, 2048) f32

Sharding: expert/channel parallel - core i computes channels [4i, 4i+4).
Each core runs 4 independent GEMMs of (256x2048)@(2048x2048) with the
contraction dim S on SBUF partitions, in fp16 (values are O(1), so fp16
gives ~3.6e-4 rel l2 error at full matmul rate and half the fp32 HBM
traffic).

Per-core roofline: PE ~110.5us (512 N=512 fp16 matmuls at ~216ns warm),
DMA ~117us (42 MB at ~358 GB/s per-core HBM share). DMA is the binding
floor, so the schedule keeps HBM saturated end to end:

- Host pre-transposes to per-partition-contiguous layouts (8 KB DMA
  descriptors): xt[c, p, k*B+b] and w[c, p, k, e].
- W streams as 1 MB (2 k-tile) chunks alternating across BOTH HWDGE
  rings (SP/sync + ACT/scalar), with ~12 MB of SBUF lookahead to ride
  through HBM arbitration jitter. Channel 0 ramps with 0.5 MB chunks
  (first one split in E halves) so the PE starts ~7us in.
- x slabs for channels 1-3 prefetch on the SWDGE (gpsimd) ring one
  channel ahead, gated behind a mid-channel W chunk so they can't
  starve the live W stream; channel 0's x rides the HWDGE rings at t=0.
- Outputs for channels 0-2 leave on the SWDGE ring (keeps the HWDGE
  FIFOs pure-W); the last channel's outputs leave eagerly per 512-col
  subtile on sync/scalar to shorten the tail.
- Eviction: VectorE adds the partition-broadcast bias (freeing the
  PSUM bank), ScalarE applies ReLU + fp16 cast. ttmp bufs=8 so the
  adds never serialize behind the activations.
"""

import os
import sys

for _p in ("/opt/trn_rl_repo", "/root/.axon_site/_ro/trn_rl_repo"):
    if os.path.isdir(_p) and _p not in sys.path:
        sys.path.insert(0, _p)

import numpy as np

import concourse.bacc as bacc
import concourse.mybir as mybir
from concourse import tile
from concourse.bass_utils import run_bass_kernel_spmd
from concourse.tile_rust import add_dep_helper

B, S, C, E = 256, 2048, 32, 2048
NCORES = 8
CPC = C // NCORES          # channels per core = 4
P = 128
KT = S // P                # 16 k-tiles
NBT = B // P               # 2 batch tiles
FREE = 512                 # matmul moving free dim (one PSUM bank of f32)
NET = E // FREE            # 4 e-tiles
KC = 2                     # k-tiles per W DMA chunk (1 MB chunks)
WBUFS = 14                 # W chunk lookahead (14 MB SBUF)
NWARM = 36                 # PE warmup matmuls (N=128) during the DMA head

_nc_cache = {}


def _build():
    f16 = mybir.dt.float16
    f32 = mybir.dt.float32
    nc = bacc.Bacc(None, target_bir_lowering=False)
    # xt[c, p, k*B + b] = x[b, k*P + p, c] : 8 KB contiguous per partition
    xt = nc.dram_tensor("xt", [CPC, P, KT * B], f16, kind="ExternalInput")
    # w[c, p, k, e] = W[c, k*P + p, e] : k-tile rows contiguous per partition
    w = nc.dram_tensor("w", [CPC, P, KT, E], f16, kind="ExternalInput")
    bias = nc.dram_tensor("bias", [CPC, E], f32, kind="ExternalInput")
    out = nc.dram_tensor("out", [B, CPC, E], f16, kind="ExternalOutput")

    with tile.TileContext(nc) as tc:
        with (
            tc.tile_pool(name="const", bufs=1) as const,
            tc.tile_pool(name="xpool", bufs=2) as xpool,
            tc.tile_pool(name="bpool", bufs=2) as bpool,
            tc.tile_pool(name="bbpool", bufs=2) as bbpool,
            tc.tile_pool(name="ttmp", bufs=8) as ttmp,
            tc.tile_pool(name="wpool", bufs=WBUFS) as wpool,
            tc.tile_pool(name="opool", bufs=4) as opool,
            tc.tile_pool(name="psum", bufs=NBT * NET, space="PSUM") as psum,
        ):
            zbias = const.tile([P, 1], f32)
            nc.any.memset(zbias[:], 0.0)
            wz = const.tile([P, P], f16)
            nc.vector.memset(wz[:], 0.0)

            xtiles: dict[int, object] = {}
            btiles: dict[int, object] = {}

            def bias_broadcast(c):
                bsb = bpool.tile([1, E], f32, name="bsb")
                nc.gpsimd.dma_start(bsb[:], bias[c : c + 1, :])
                bbc = bbpool.tile([P, E], f32, name="bbc")
                nc.gpsimd.partition_broadcast(bbc[:], bsb[:])
                btiles[c] = bbc

            # Channel 0's x loads in 3 pieces: k0-1 races the first W chunk
            # on the scalar ring (sync carries pure W), the rest rides the
            # otherwise-idle SWDGE ring. The first matmuls only need piece A.
            xsb0 = xpool.tile([P, KT * B], f16, name="xsb")
            nc.scalar.dma_start(xsb0[:, : 2 * B], xt[0, :, : 2 * B])
            nc.gpsimd.dma_start(xsb0[:, 2 * B : 8 * B], xt[0, :, 2 * B : 8 * B])
            nc.gpsimd.dma_start(xsb0[:, 8 * B :], xt[0, :, 8 * B :])
            xtiles[0] = xsb0
            bias_broadcast(0)



            def prefetch_channel(c, after):
                # next channel's x slab + bias on the SWDGE ring, held back
                # until mid-channel so it doesn't steal HBM bandwidth from
                # the live W stream (GpSimd is in-order: gating the slab
                # gates everything behind it too)
                xsb = xpool.tile([P, KT * B], f16, name="xsb")
                xdma = nc.gpsimd.dma_start(xsb[:], xt[c, :, :])
                add_dep_helper(
                    xdma.ins,
                    after.ins,
                    reason="x prefetch waits for mid-channel W chunk",
                )
                xtiles[c] = xsb
                bias_broadcast(c)

            qtog = [0]

            def weng():
                qtog[0] ^= 1
                return nc.sync if qtog[0] else nc.scalar

            for c in range(CPC):
                xsb = xtiles[c]
                ps = [
                    [
                        psum.tile([P, FREE], f32, name="ps")
                        for _ in range(NET)
                    ]
                    for _ in range(NBT)
                ]
                if c == 0:
                    # PE warmup: ~3.6us of zero matmuls into the first PSUM
                    # bank while the first x/W DMAs are in flight, so the
                    # HAM clock gate reaches 8/8 before the real matmuls
                    # start (otherwise the first ~35 run at 1.2 GHz and any
                    # ramp-delivery gap restarts the warmup window). The
                    # real k0 matmul (start=True) overwrites the garbage.
                    for _ in range(NWARM):
                        nc.tensor.matmul(
                            ps[0][0][:, :P], wz[:], wz[:], start=True, stop=True
                        )
                # W chunk schedule: channel 0 ramps with 1-k-tile chunks for
                # k0-k7 so, with strict per-ring FIFO ordering and queue
                # alternation, W k-tiles arrive in need order at most one
                # tile ahead; later channels stream 2-k-tile chunks.
                chunk_kts = [1] * 8 + [KC] * 4 if c == 0 else [KC] * (KT // KC)
                k = 0
                prefetched = False
                for ci, ckt in enumerate(chunk_kts):
                    wsb = wpool.tile([P, KC, E], f16, name="wsb")
                    eng = weng()
                    if c == 0 and ci == 0:
                        # split the very first chunk in E halves so the
                        # et0/et1 matmuls start ~0.7us sooner
                        eng.dma_start(
                            wsb[:, :1, : E // 2], w[c, :, k : k + 1, : E // 2]
                        )
                        wdma = eng.dma_start(
                            wsb[:, :1, E // 2 :], w[c, :, k : k + 1, E // 2 :]
                        )
                    else:
                        wdma = eng.dma_start(
                            wsb[:, :ckt, :], w[c, :, k : k + ckt, :]
                        )
                    for kk in range(ckt):
                        for bt in range(NBT):
                            lhsT = xsb[:, k * B + bt * P : k * B + (bt + 1) * P]
                            for et in range(NET):
                                nc.tensor.matmul(
                                    ps[bt][et][:],
                                    lhsT,
                                    wsb[:, kk, et * FREE : (et + 1) * FREE],
                                    start=(k == 0),
                                    stop=(k == KT - 1),
                                )
                        k += 1
                    if not prefetched and k >= 6 and c + 1 < CPC:
                        prefetch_channel(c + 1, after=wdma)
                        prefetched = True
                # Evict: VectorE adds the broadcast bias (freeing the PSUM
                # bank), ScalarE applies ReLU + fp16 cast.
                bbc = btiles[c]
                last = c == CPC - 1
                oq = [0]
                for bt in range(NBT):
                    ot = opool.tile([P, E], f16)
                    for et in range(NET):
                        dst = ot[:, et * FREE : (et + 1) * FREE]
                        # fp16 tmp: halves ttmp SBUF; the pre-activation is
                        # O(1) so the fp16 rounding matches the final cast
                        tmp = ttmp.tile([P, FREE], f16, name="tmp")
                        nc.vector.tensor_add(
                            tmp[:],
                            ps[bt][et][:],
                            bbc[:, et * FREE : (et + 1) * FREE],
                        )
                        nc.scalar.activation(
                            dst,
                            tmp[:],
                            mybir.ActivationFunctionType.Relu,
                            bias=zbias[:],
                        )
                        if last:
                            # tail: eager per-subtile DMAs spread over all
                            # three rings (the W stream is finished by now);
                            # gpsimd takes the first two so its ~2us SWDGE
                            # setup cost overlaps the remaining evictions
                            oengs = [
                                nc.gpsimd, nc.gpsimd, nc.sync, nc.scalar,
                                nc.sync, nc.scalar, nc.sync, nc.scalar,
                            ]
                            oeng = oengs[oq[0]]
                            oq[0] += 1
                            oeng.dma_start(
                                out[
                                    bt * P : (bt + 1) * P,
                                    c,
                                    et * FREE : (et + 1) * FREE,
                                ],
                                dst,
                            )
                    if not last:
                        # one 1 MB DMA per (bt, c) on the SWDGE ring,
                        # keeping both HWDGE rings pure-W
                        nc.gpsimd.dma_start(out[bt * P : (bt + 1) * P, c, :], ot[:])
    nc.compile()
    return nc


def _get_nc():
    if "nc" not in _nc_cache:
        _nc_cache["nc"] = _build()
    return _nc_cache["nc"]


def _run(x, W, b, **spmd_kwargs):
    nc = _get_nc()

    in_maps = []
    for i in range(NCORES):
        c0, c1 = i * CPC, (i + 1) * CPC
        # xt[c, p, k*B + b] = x[b, k*P + p, c]
        xs = x[:, :, c0:c1].astype(np.float16)           # (B, S, CPC)
        xs = xs.transpose(2, 1, 0).reshape(CPC, KT, P, B)
        xt_i = np.ascontiguousarray(xs.transpose(0, 2, 1, 3)).reshape(
            CPC, P, KT * B
        )
        # w[c, p, k, e] = W[c, k*P + p, e]
        ws = W[c0:c1].astype(np.float16).reshape(CPC, KT, P, E)
        w_i = np.ascontiguousarray(ws.transpose(0, 2, 1, 3))
        b_i = np.ascontiguousarray(b[c0:c1].astype(np.float32))
        in_maps.append({"xt": xt_i, "w": w_i, "bias": b_i})

    res = run_bass_kernel_spmd(
        nc, in_maps, core_ids=list(range(NCORES)), **spmd_kwargs
    )
    out = np.concatenate(
        [r["out"].astype(np.float32) for r in res.results], axis=1
    )
    return out, res


def kernel(x: np.ndarray, W: np.ndarray, b: np.ndarray) -> np.ndarray:
    out, _ = _run(x, W, b)
    return out
